# revision 19
# baseline (speedup 1.0000x reference)
"""Trainium2 Bass kernel for nn_EvoformerNoTriMul (B=2, L=384, H=256, P=64, NH=4,
4 layers x 3 recycles), sharded over 8 NeuronCores by pair rows (48 i-rows each).

Pair tensor lives SBUF-resident feature-major: [128 partitions = (b*64+f),
48*384 tokens (io,j)], fp32. All LayerNorm mean terms are folded into modified
weight matrices host-side; variances come from rank-decompositions + tiny
matmuls; per-token inv-std factors are applied via DMA-broadcast bf16 tiles.
Sequence track is row-sharded with a per-layer AllGather; recycle
symmetrization uses AllToAll; final output is symmetrized on the host.
"""

import sys

import numpy as np

sys.path.insert(0, "/opt/trn_rl_repo")

import ml_dtypes  # noqa: E402

B, L, AA, H, P, NH, NB = 2, 384, 48, 256, 64, 4, 64
HD = H // NH
NUM_LAYERS = 4
N_REC = 3
MAX_REL = 32
NC = 8
LC = L // NC            # 48
NT = LC * L             # 18432 pair tokens per core (free dim)
CH = 512                # pair stream chunk
NCH = NT // CH          # 36
GRP = 4                 # slabs per gelu group
SL = GRP * L            # 1536
NGRP = LC // GRP        # 12
EPS = 1e-5

_CACHE = {}


# ============================================================================
# Host-side folding
# ============================================================================

def _fold_params(params):
    f = {}
    g32 = lambda a: np.asarray(a, np.float32)
    p = params
    for k in ("Wres", "bres", "pos", "Wpi", "bpi", "Wpj", "bpj", "rel"):
        f[k] = g32(p[k])
    layers = []
    for lp in p["layers"]:
        q = {k: g32(v) for k, v in lp.items()}
        d = {}
        d["Wq"] = (q["ns_g"][:, None] * q["Wq"]) * (HD ** -0.5)
        d["Wq_c"] = (q["ns_b"] @ q["Wq"]) * (HD ** -0.5)
        d["Wk"] = q["ns_g"][:, None] * q["Wk"]
        d["Wk_c"] = q["ns_b"] @ q["Wk"]
        d["Wv"] = q["ns_g"][:, None] * q["Wv"]
        d["Wv_c"] = q["ns_b"] @ q["Wv"]
        d["Wo"], d["bo"] = q["Wo"], q["bo"]
        d["Wbp"] = q["np_g"][:, None] * q["Wb"]
        d["wbs"] = d["Wbp"].sum(0)
        d["cb"] = q["np_b"] @ q["Wb"]
        d["W1"] = q["ff_g"][:, None] * q["W1"]
        d["W1_c"] = q["ff_b"] @ q["W1"] + q["b1"]
        d["W2"], d["b2"] = q["W2"], q["b2"]
        d["Wi"], d["Wj"] = q["Wout"][:H], q["Wout"][H:]
        d["bout"] = q["bout"]
        W1p = q["pu_g"][:, None] * q["pW1"]
        w1s = W1p.sum(0)
        d["M1"] = W1p - np.full((P, 1), 1.0 / P, np.float32) * w1s[None, :]
        d["bb"] = q["pu_b"] @ q["pW1"] + q["pb1"]
        d["pW2"], d["pb2"] = q["pW2"], q["pb2"]
        layers.append(d)
    f["layers"] = layers
    dW1p = g32(p["dg_g"])[:, None] * g32(p["dW1"])
    dw1s = dW1p.sum(0)
    f["dM1"] = dW1p - np.full((P, 1), 1.0 / P, np.float32) * dw1s[None, :]
    f["dbb"] = g32(p["dg_b"]) @ g32(p["dW1"]) + g32(p["db1"])
    f["dW2"] = g32(p["dW2"])
    f["db2"] = g32(p["db2"])
    f["dW2r"] = f["dW2"] @ g32(p["Wrec"]) * 0.5
    f["drc"] = f["db2"] @ g32(p["Wrec"]) + g32(p["brec"])
    return f


def _host_precompute(x, f):
    hp = {}
    x = np.asarray(x, np.float32)
    h = x @ f["Wres"] + f["bres"] + f["pos"][:L][None]
    hp["h_init"] = h
    hp["rowp0"] = h @ f["Wpi"] + f["bpi"]
    hp["colp0"] = h @ f["Wpj"] + f["bpj"]
    k = np.arange(2 * L - 1)
    idx = np.clip(k - (L - 1), -MAX_REL, MAX_REL) + MAX_REL
    hp["T"] = f["rel"][idx]          # [767, P]
    return hp


def _bp(a):
    """[B, n, P] -> [128, n] batch-packed feature-major."""
    return np.ascontiguousarray(
        np.concatenate([a[0].T, a[1].T], axis=0), dtype=np.float32)


def _bf(a):
    return np.ascontiguousarray(a).astype(ml_dtypes.bfloat16)


# ============================================================================
# Device program
# ============================================================================

def _build_program(f):
    import concourse.bacc as bacc
    import concourse.tile as tile
    from concourse import mybir

    nc = bacc.Bacc(trn_type="TRN2", num_devices=NC)
    dt = mybir.dt

    din = {}

    def inp(name, shape, dtype=dt.float32):
        din[name] = nc.dram_tensor(name, list(shape), dtype, kind="ExternalInput")

    inp("h0", [96, NC + 1, H])
    inp("rowp0", [128, LC])
    inp("colp0", [128, L])
    inp("Tc", [128, LC - 1 + L])
    for li in range(NUM_LAYERS):
        inp(f"wq_{li}", [H, H], dt.bfloat16)
        inp(f"wk_{li}", [H, H], dt.bfloat16)
        inp(f"wv_{li}", [H, H], dt.bfloat16)
        inp(f"wo_{li}", [H, H], dt.bfloat16)
        inp(f"w1_{li}", [H, 4 * H], dt.bfloat16)
        inp(f"w2_{li}", [4 * H, H], dt.bfloat16)
        inp(f"wi_{li}", [H, P], dt.bfloat16)
        inp(f"wj_{li}", [H, P], dt.bfloat16)
        inp(f"m1_{li}", [128, P], dt.bfloat16)
        inp(f"pw2_{li}", [128, P], dt.bfloat16)
        inp(f"biasS_{li}", [128, 8], dt.bfloat16)
        inp(f"w1c_{li}", [128, 8])
        inp(f"vecs_{li}", [128, 8])     # col0 pb2, col1 bout
    inp("dm1", [128, P], dt.bfloat16)
    inp("dw2", [128, P], dt.bfloat16)
    inp("dw2r", [128, P], dt.bfloat16)
    inp("dvec", [128, 4])               # col0 drc
    inp("onesbp", [128, 1], dt.bfloat16)

    t_out = nc.dram_tensor("t_out", [128, NT], dt.float32, kind="ExternalOutput")

    with tile.TileContext(nc) as tc:
        _emit(tc, nc, f, din, t_out, tile, mybir)
    nc.finalize()
    return nc


def _emit(tc, nc, f, din, t_out, tile, mybir):
    import contextlib

    import concourse.bass as bass
    from concourse.masks import make_identity

    dt = mybir.dt
    AF = mybir.ActivationFunctionType
    ALU = mybir.AluOpType
    f32, f32r, bf16 = dt.float32, dt.float32r, dt.bfloat16
    RG = [list(range(NC))]

    assert all(np.allclose(d["bb"], 0) for d in f["layers"]), "bb != 0"
    assert np.allclose(f["dbb"], 0), "dbb != 0"

    def pbc(ap_row, n):
        """[1, F] slice -> [n, F] partition-broadcast AP (for DMA sources)."""
        assert ap_row.shape[0] == 1
        return bass.AP(tensor=ap_row.tensor, offset=ap_row.offset,
                       ap=[[0, n]] + [list(d) for d in ap_row.ap[1:]])

    ctx = contextlib.ExitStack()
    perm = ctx.enter_context(tc.tile_pool(name="perm", bufs=1))
    wpool = ctx.enter_context(tc.tile_pool(name="wts", bufs=1))
    run = ctx.enter_context(tc.tile_pool(name="run", bufs=2))
    run2 = ctx.enter_context(tc.tile_pool(name="run2", bufs=1))
    smal = ctx.enter_context(tc.tile_pool(name="smal", bufs=2))
    ps = ctx.enter_context(tc.tile_pool(name="ps", bufs=2, space="PSUM"))
    ps2 = ctx.enter_context(tc.tile_pool(name="ps2", bufs=3, space="PSUM"))
    ps3 = ctx.enter_context(tc.tile_pool(name="ps3", bufs=1, space="PSUM"))
    dram = ctx.enter_context(tc.tile_pool(name="dram", bufs=1, space="DRAM"))

    # ---------------- persistent state ----------------
    pair = perm.tile([128, NT], f32)
    s_sb = perm.tile([96, NC + 1, H], f32)
    idn = perm.tile([128, 128], bf16)
    make_identity(nc, idn)
    epsv = perm.tile([128, 1], f32)
    nc.vector.memset(epsv, EPS)

    nc.sync.dma_start(s_sb, din["h0"][:, :, :])

    def load_const(name, shape, dtype):
        t = perm.tile(shape, dtype, tag=name)
        nc.sync.dma_start(t, din[name][:, :])
        return t

    onesbp = load_const("onesbp", [128, 1], bf16)
    dm1_sb = load_const("dm1", [128, P], bf16)
    dw2_sb = load_const("dw2", [128, P], bf16)
    dw2r_sb = load_const("dw2r", [128, P], bf16)
    dvec_sb = load_const("dvec", [128, 4], f32)

    lw = []
    for li in range(NUM_LAYERS):
        d = {}
        for nm, shp, dty in (
            ("m1", [128, P], bf16), ("pw2", [128, P], bf16),
            ("biasS", [128, 8], bf16), ("w1c", [128, 8], f32),
            ("vecs", [128, 8], f32),
        ):
            d[nm] = load_const(f"{nm}_{li}", shp, dty)
        lw.append(d)

    rowp0 = load_const("rowp0", [128, LC], f32)
    colp0 = load_const("colp0", [128, L], f32)
    Tcs = load_const("Tc", [128, LC - 1 + L], f32)

    for io in range(LC):
        nc.vector.scalar_tensor_tensor(
            out=pair[:, io * L:(io + 1) * L],
            in0=Tcs[:, LC - 1 - io:LC - 1 - io + L],
            scalar=rowp0[:, io:io + 1],
            in1=colp0[:, :],
            op0=ALU.add, op1=ALU.add,
        )

    bs_dram = dram.tile([128, NT], bf16)
    stc_dram = dram.tile([4, L], f32)
    istd_dram = dram.tile([2, NT], bf16)
    ag_in = dram.tile([96, H], f32)
    ag_outs = [dram.tile([96 * NC, H], f32, addr_space="Shared",
                         tag=f"agout{i}", name=f"agout{i}")
               for i in range(N_REC * NUM_LAYERS)]
    r_dram = dram.tile([128, NT], bf16)
    a2a_in = dram.tile([NC, 128, LC * LC], bf16)
    a2a_out = dram.tile([NC, 128, LC * LC], bf16)

    # ---------------- helpers ----------------
    def ln_own(src_ap, dst):
        st = smal.tile([96, 6], f32, tag="lnst")
        nc.vector.bn_stats(st, src_ap)
        mv = smal.tile([96, 2], f32, tag="lnmv")
        nc.vector.bn_aggr(mv, st)
        sd = smal.tile([96, 1], f32, tag="lnsd")
        nc.scalar.activation(sd, mv[:, 1:2], AF.Sqrt, bias=epsv[0:96, :], scale=1.0)
        nc.vector.reciprocal(sd, sd)
        nc.vector.tensor_scalar(
            out=dst, in0=src_ap, scalar1=mv[:, 0:1], scalar2=sd,
            op0=ALU.subtract, op1=ALU.mult)

    def stats_pass(biasS_t):
        """pair + pair^2 streamed through [Wb'|1] and ones matmuls -> bs_dram.
        psum rows: 0:5 b0 (bias,S1), 32:37 b1, 64:65 S2-b0, 96:97 S2-b1."""
        for ch in range(NCH):
            c0, c1 = ch * CH, (ch + 1) * CH
            pbf = run.tile([128, CH], bf16, tag="pbf")
            nc.vector.tensor_copy(pbf, pair[:, c0:c1])
            sq = run.tile([128, CH], bf16, tag="sq")
            nc.vector.tensor_tensor(out=sq, in0=pbf, in1=pbf, op=ALU.mult)
            st = ps.tile([128, CH], f32, tag="big")
            nc.tensor.matmul(st[0:5, :], biasS_t[0:64, 0:5], pbf[0:64, :],
                             tile_position=(0, 0))
            nc.tensor.matmul(st[32:37, :], biasS_t[64:128, 0:5], pbf[64:128, :],
                             tile_position=(64, 32))
            nc.tensor.matmul(st[64:65, :], onesbp[0:64, :], sq[0:64, :],
                             tile_position=(0, 64))
            nc.tensor.matmul(st[96:97, :], onesbp[64:128, :], sq[64:128, :],
                             tile_position=(64, 96))
            bsx = run.tile([128, CH], bf16, tag="bsx")
            nc.vector.tensor_copy(bsx, st)
            nc.sync.dma_start(bs_dram[:, c0:c1], bsx)

    def row_to_tm(src, row, tag, dtype=bf16):
        """bs_dram-style row [1, NT] -> [LC, L] tile via DMA reshape."""
        t = smal.tile([LC, L], dtype, tag=tag)
        nc.sync.dma_start(t, src[row:row + 1, :])
        return t

    def istd_from_stats(b, want_fix, want_dram):
        s1 = row_to_tm(bs_dram, 4 if b == 0 else 36, "s1row")
        s2 = row_to_tm(bs_dram, 64 if b == 0 else 96, "s2row")
        m = smal.tile([LC, L], f32, tag="mtm", bufs=1)
        nc.vector.tensor_scalar_mul(out=m, in0=s1, scalar1=1.0 / P)
        m2 = smal.tile([LC, L], f32, tag="m2tm", bufs=1)
        nc.vector.tensor_tensor(out=m2, in0=m, in1=m, op=ALU.mult)
        nc.vector.scalar_tensor_tensor(out=m2, in0=s2, scalar=1.0 / P,
                                       in1=m2, op0=ALU.mult, op1=ALU.subtract)
        nc.scalar.activation(m2, m2, AF.Sqrt, bias=epsv[0:LC, :], scale=1.0)
        istd = smal.tile([LC, L], f32, tag="istdtm")
        nc.vector.reciprocal(istd, m2)
        e = None
        if want_fix:
            e = smal.tile([LC, L], f32, tag="etm")
            nc.vector.scalar_tensor_tensor(out=e, in0=m, scalar=-1.0, in1=istd,
                                           op0=ALU.mult, op1=ALU.mult)
        if want_dram:
            ib = smal.tile([LC, L], bf16, tag="istdbf", bufs=1)
            nc.vector.tensor_copy(ib, istd)
            nc.sync.dma_start(istd_dram[b:b + 1, :], ib)
        return istd, e

    def tr96(src_slice, dst_slice):
        """transpose [96,128] bf16 -> psum [128,96] -> copy to dst."""
        pt = ps3.tile([128, 96], bf16, tag="tr")
        nc.tensor.transpose(pt, src_slice, idn[0:96, 0:96])
        nc.vector.tensor_copy(dst_slice, pt)

    def tr96_bsplit(src_slice, dst_fm, kc, s):
        pt = ps3.tile([128, 96], bf16, tag="tr")
        nc.tensor.transpose(pt, src_slice, idn[0:96, 0:96])
        nc.vector.tensor_copy(dst_fm[:, kc, s * LC:(s + 1) * LC], pt[:, 0:LC])
        nc.vector.tensor_copy(dst_fm[:, kc, L + s * LC:L + (s + 1) * LC],
                              pt[:, LC:96])

    def mm_k2(psum, w_t, rhs_fm_slices, mo):
        for kc in range(2):
            nc.tensor.matmul(psum, w_t[:, kc, mo * 128:(mo + 1) * 128],
                             rhs_fm_slices[kc], start=(kc == 0), stop=(kc == 1))

    def quad_mm(out_ps, lhs_t, rhs_t, lslice=slice(0, P)):
        nc.tensor.matmul(out_ps[0:64, :], lhs_t[0:64, lslice], rhs_t[0:64, :],
                         tile_position=(0, 0))
        nc.tensor.matmul(out_ps[64:128, :], lhs_t[64:128, lslice],
                         rhs_t[64:128, :], tile_position=(64, 64))

    def sum_quads(out_ps, vec_t, sq_t):
        nc.tensor.matmul(out_ps[0:1, :], onesbp[0:64, :], vec_t[0:64, :],
                         tile_position=(0, 0))
        nc.tensor.matmul(out_ps[32:33, :], onesbp[64:128, :], vec_t[64:128, :],
                         tile_position=(64, 32))
        nc.tensor.matmul(out_ps[64:65, :], onesbp[0:64, :], sq_t[0:64, :],
                         tile_position=(0, 64))
        nc.tensor.matmul(out_ps[96:97, :], onesbp[64:128, :], sq_t[64:128, :],
                         tile_position=(64, 96))

    # ================================================================
    for cyc in range(N_REC):
        if cyc > 0:
            nc.sync.dma_start(s_sb, din["h0"][:, :, :])
        for li in range(NUM_LAYERS):
            d = lw[li]
            fl = f["layers"][li]
            wd = {}
            for nm, kk, m in (("wq", 2, H), ("wk", 2, H), ("wv", 2, H),
                              ("wo", 2, H), ("w1", 2, 4 * H), ("w2", 8, H),
                              ("wi", 2, P), ("wj", 2, P)):
                t = wpool.tile([128, kk, m], bf16, tag=nm)
                nc.sync.dma_start(
                    t, din[f"{nm}_{li}"][:, :].rearrange("(c p) m -> p c m", p=128))
                wd[nm] = t

            # ---- phase A: pair stats + attention bias ----
            stats_pass(d["biasS"])
            fx = {}
            for b in range(2):
                fx[b] = istd_from_stats(b, want_fix=True, want_dram=False)
            bias_tm = {}
            for b in range(2):
                for h in range(NH):
                    bias_tm[(b, h)] = row_to_tm(bs_dram, (0 if b == 0 else 32) + h, f"btm{h}")

            # ---- phase B: attention ----
            sn_own = smal.tile([96, H], bf16, tag="snown")
            ln_own(s_sb[:, 0, :], sn_own)
            st8 = smal.tile([96, NC, 6], f32, tag="st8")
            mv8 = smal.tile([96, NC, 2], f32, tag="mv8")
            for s in range(NC):
                nc.vector.bn_stats(st8[:, s, :], s_sb[:, 1 + s, :])
                nc.vector.bn_aggr(mv8[:, s, :], st8[:, s, :])
            sd8 = smal.tile([96, NC], f32, tag="sd8")
            nc.scalar.activation(sd8, mv8[:, :, 1], AF.Sqrt, bias=epsv[0:96, :], scale=1.0)
            nc.vector.reciprocal(sd8, sd8)
            sn_all = run2.tile([96, NC, H], bf16, tag="s96bf")
            for s in range(NC):
                nc.vector.tensor_scalar(
                    out=sn_all[:, s, :], in0=s_sb[:, 1 + s, :],
                    scalar1=mv8[:, s, 0:1], scalar2=sd8[:, s:s + 1],
                    op0=ALU.subtract, op1=ALU.mult)

            snall_fm = run2.tile([128, 2, 2 * L], bf16, tag="fm768")
            for s in range(NC):
                for kc in range(2):
                    tr96_bsplit(sn_all[:, s, kc * 128:(kc + 1) * 128],
                                snall_fm, kc, s)
            snown_fm = smal.tile([128, 2, 96], bf16, tag="snownfm")
            for kc in range(2):
                tr96(sn_own[:, kc * 128:(kc + 1) * 128], snown_fm[:, kc, :])

            q_fm = smal.tile([128, 2, 96], bf16, tag="qfm")
            for mo in range(2):
                qp = ps2.tile([128, 96], f32, tag="mid")
                mm_k2(qp, wd["wq"], [snown_fm[:, 0, :], snown_fm[:, 1, :]], mo)
                nc.vector.tensor_copy(q_fm[:, mo, :], qp)
            k_fm = run2.tile([128, 2, 2 * L], bf16, tag="kfm")
            v_fm = run2.tile([128, 2, 2 * L], bf16, tag="vfm")
            for dst, wt in ((k_fm, "wk"), (v_fm, "wv")):
                for mo in range(2):
                    for nh in range(2):
                        kp = ps2.tile([128, L], f32, tag="mid")
                        mm_k2(kp, wd[wt],
                              [snall_fm[:, 0, nh * L:(nh + 1) * L],
                               snall_fm[:, 1, nh * L:(nh + 1) * L]], mo)
                        nc.vector.tensor_copy(dst[:, mo, nh * L:(nh + 1) * L], kp)
            v_tm = run2.tile([128, 6, H], bf16, tag="vtm")
            for mo in range(2):
                for chk in range(6):
                    pt = ps3.tile([128, 128], bf16, tag="tr")
                    nc.tensor.transpose(pt, v_fm[:, mo, chk * 128:(chk + 1) * 128],
                                        idn)
                    nc.vector.tensor_copy(v_tm[:, chk, mo * 128:(mo + 1) * 128], pt)

            o_fm = smal.tile([128, 2, 96], bf16, tag="ofm")
            for b in range(2):
                istd_tm, e_tm = fx[b]
                for h in range(NH):
                    mo, pr = h // 2, (h % 2) * 64
                    scp = ps2.tile([LC, L], f32, tag="mid")
                    nc.tensor.matmul(
                        scp, q_fm[pr:pr + 64, mo, b * LC:(b + 1) * LC],
                        k_fm[pr:pr + 64, mo, b * L:(b + 1) * L],
                        tile_position=(pr, 0))
                    x1 = smal.tile([LC, L], bf16, tag="x1")
                    nc.vector.tensor_tensor(out=x1, in0=bias_tm[(b, h)],
                                            in1=istd_tm, op=ALU.mult)
                    x2 = smal.tile([LC, L], bf16, tag="x2")
                    nc.vector.scalar_tensor_tensor(
                        out=x2, in0=e_tm, scalar=float(fl["wbs"][h]),
                        in1=x1, op0=ALU.mult, op1=ALU.add)
                    sc = smal.tile([LC, L], f32, tag="scsb", bufs=1)
                    nc.vector.scalar_tensor_tensor(
                        out=sc, in0=scp, scalar=float(fl["cb"][h]),
                        in1=x2, op0=ALU.add, op1=ALU.add)
                    nmx = smal.tile([LC, 1], f32, tag="nmx")
                    nc.vector.tensor_reduce(nmx, sc, axis=mybir.AxisListType.X,
                                            op=ALU.max, negate=True)
                    pa = smal.tile([LC, L], bf16, tag="pattn")
                    sxp = smal.tile([LC, 1], f32, tag="sxp")
                    nc.scalar.activation(pa, sc, AF.Exp, bias=nmx, scale=1.0,
                                         accum_out=sxp)
                    nc.vector.reciprocal(sxp, sxp)
                    nc.vector.tensor_scalar_mul(out=pa, in0=pa, scalar1=sxp)
                    pT = smal.tile([128, 3, LC], bf16, tag="pT")
                    for chk in range(3):
                        pt = ps3.tile([128, LC], bf16, tag="tr")
                        nc.tensor.transpose(pt, pa[:, chk * 128:(chk + 1) * 128],
                                            idn[0:LC, 0:LC])
                        nc.vector.tensor_copy(pT[:, chk, :], pt)
                    op_ = ps2.tile([64, LC], f32, tag="mid")
                    for chk in range(3):
                        nc.tensor.matmul(
                            op_, v_tm[:, b * 3 + chk, h * 64:(h + 1) * 64],
                            pT[:, chk, :], start=(chk == 0), stop=(chk == 2))
                    nc.vector.tensor_copy(
                        o_fm[pr:pr + 64, mo, b * LC:(b + 1) * LC], op_)

            for mo in range(2):
                owp = ps2.tile([128, 96], f32, tag="mid")
                mm_k2(owp, wd["wo"], [o_fm[:, 0, :], o_fm[:, 1, :]], mo)
                ow = smal.tile([128, 96], bf16, tag="owsb")
                nc.vector.tensor_copy(ow, owp)
                pt = ps3.tile([96, 128], bf16, tag="trpsT")
                nc.tensor.transpose(pt, ow, idn)
                nc.vector.tensor_tensor(
                    out=s_sb[:, 0, mo * 128:(mo + 1) * 128],
                    in0=s_sb[:, 0, mo * 128:(mo + 1) * 128], in1=pt, op=ALU.add)

            # ---- FF ----
            sf = smal.tile([96, H], bf16, tag="sfln")
            ln_own(s_sb[:, 0, :], sf)
            sf_fm = smal.tile([128, 2, 96], bf16, tag="sffm")
            for kc in range(2):
                tr96(sf[:, kc * 128:(kc + 1) * 128], sf_fm[:, kc, :])
            g1 = run2.tile([128, 8, 96], bf16, tag="g1")
            for mo in range(8):
                fp = ps2.tile([128, 96], f32, tag="mid")
                mm_k2(fp, wd["w1"], [sf_fm[:, 0, :], sf_fm[:, 1, :]], mo)
                nc.scalar.activation(g1[:, mo, :], fp, AF.Gelu,
                                     bias=d["w1c"][:, mo:mo + 1], scale=1.0)
            for mo in range(2):
                fo = ps2.tile([128, 96], f32, tag="mid")
                for kc in range(8):
                    nc.tensor.matmul(fo, wd["w2"][:, kc, mo * 128:(mo + 1) * 128],
                                     g1[:, kc, :], start=(kc == 0), stop=(kc == 7))
                fs = smal.tile([128, 96], bf16, tag="fosb")
                nc.vector.tensor_copy(fs, fo)
                pt = ps3.tile([96, 128], bf16, tag="trpsT")
                nc.tensor.transpose(pt, fs, idn)
                nc.vector.tensor_tensor(
                    out=s_sb[:, 0, mo * 128:(mo + 1) * 128],
                    in0=s_sb[:, 0, mo * 128:(mo + 1) * 128], in1=pt, op=ALU.add)

            # ---- AllGather s ----
            ag_out = ag_outs[cyc * NUM_LAYERS + li]
            nc.sync.dma_start(ag_in[:, :], s_sb[:, 0, :])
            nc.gpsimd.collective_compute(
                "AllGather", ALU.bypass, replica_groups=RG,
                ins=[ag_in[:, :]], outs=[ag_out[:, :]])
            nc.sync.dma_start(
                s_sb[:, 1:, :],
                ag_out[:, :].rearrange("(c p) h -> p c h", p=96))

            # ---- phase D: pair projections ----
            s_bf = run2.tile([96, NC, H], bf16, tag="s96bf")
            nc.vector.tensor_copy(s_bf[:, :, :], s_sb[:, 1:, :])
            s_fm = run2.tile([128, 2, 2 * L], bf16, tag="fm768")
            for s in range(NC):
                for kc in range(2):
                    tr96_bsplit(s_bf[:, s, kc * 128:(kc + 1) * 128], s_fm, kc, s)
            sown_bf = smal.tile([96, H], bf16, tag="sownbf")
            nc.vector.tensor_copy(sown_bf, s_sb[:, 0, :])
            sown_fm = smal.tile([128, 2, 96], bf16, tag="sownfm")
            for kc in range(2):
                tr96(sown_bf[:, kc * 128:(kc + 1) * 128], sown_fm[:, kc, :])

            colp = run2.tile([128, L], bf16, tag="colp")
            for nh in range(2):
                cp = ps2.tile([64, L], f32, tag="mid")
                for kc in range(2):
                    nc.tensor.matmul(cp, wd["wj"][:, kc, :],
                                     s_fm[:, kc, nh * L:(nh + 1) * L],
                                     start=(kc == 0), stop=(kc == 1))
                nc.vector.tensor_copy(colp[nh * 64:(nh + 1) * 64, :], cp)
            rowp = smal.tile([128, LC], bf16, tag="rowp")
            rp = ps2.tile([64, 96], f32, tag="mid")
            for kc in range(2):
                nc.tensor.matmul(rp, wd["wi"][:, kc, :], sown_fm[:, kc, :],
                                 start=(kc == 0), stop=(kc == 1))
            nc.scalar.activation(rowp[0:64, :], rp[:, 0:LC], AF.Identity,
                                 bias=d["vecs"][0:64, 1:2], scale=1.0)
            nc.scalar.activation(rowp[64:128, :], rp[:, LC:96], AF.Identity,
                                 bias=d["vecs"][64:128, 1:2], scale=1.0)

            qcol = run2.tile([128, L], bf16, tag="qcol")
            qcp = ps2.tile([128, L], f32, tag="mid")
            quad_mm(qcp, d["m1"], colp)
            nc.vector.tensor_copy(qcol, qcp)
            qrow = smal.tile([128, LC], f32, tag="qrow")
            qrp = ps2.tile([128, LC], f32, tag="mid")
            quad_mm(qrp, d["m1"], rowp)
            nc.vector.tensor_copy(qrow, qrp)

            sqc = run2.tile([128, L], bf16, tag="sqc")
            nc.vector.tensor_tensor(out=sqc, in0=colp, in1=colp, op=ALU.mult)
            stc = ps.tile([128, L], f32, tag="big")
            sum_quads(stc, colp, sqc)
            stc_sb = smal.tile([128, L], f32, tag="stcsb", bufs=1)
            nc.vector.tensor_copy(stc_sb, stc)
            for i_, r_ in enumerate((0, 32, 64, 96)):
                nc.sync.dma_start(stc_dram[i_:i_ + 1, :], stc_sb[r_:r_ + 1, :])

            sqr = smal.tile([128, LC], bf16, tag="sqr")
            nc.vector.tensor_tensor(out=sqr, in0=rowp, in1=rowp, op=ALU.mult)
            strp = ps.tile([128, LC], f32, tag="big")
            sum_quads(strp, rowp, sqr)
            str_sb = smal.tile([128, LC], f32, tag="strsb")
            nc.vector.tensor_copy(str_sb, strp)

            for b in range(2):
                r0, r2 = (0, 64) if b == 0 else (32, 96)
                mr = smal.tile([LC, 1], f32, tag="mrT")
                nc.sync.dma_start(mr, str_sb[r0:r0 + 1, :])
                s2r = smal.tile([LC, 1], f32, tag="s2rT")
                nc.sync.dma_start(s2r, str_sb[r2:r2 + 1, :])
                nc.vector.tensor_scalar_mul(out=mr, in0=mr, scalar1=1.0 / P)
                mr2 = smal.tile([LC, 1], f32, tag="mr2")
                nc.vector.tensor_tensor(out=mr2, in0=mr, in1=mr, op=ALU.mult)
                varr = smal.tile([LC, 1], f32, tag="varr")
                nc.vector.scalar_tensor_tensor(
                    out=varr, in0=s2r, scalar=1.0 / P, in1=mr2,
                    op0=ALU.mult, op1=ALU.subtract)
                m2r = smal.tile([LC, 1], f32, tag="m2r")
                nc.vector.tensor_scalar_mul(out=m2r, in0=mr, scalar1=-1.0 / 32.0)
                mcB = smal.tile([LC, L], f32, tag="mcB", bufs=1)
                nc.sync.dma_start(mcB, pbc(stc_dram[b:b + 1, :], LC))
                s2cB = smal.tile([LC, L], f32, tag="s2cB", bufs=1)
                nc.sync.dma_start(s2cB, pbc(stc_dram[2 + b:3 + b, :], LC))
                mc = smal.tile([LC, L], f32, tag="mcs", bufs=1)
                nc.vector.tensor_scalar_mul(out=mc, in0=mcB, scalar1=1.0 / P)
                nc.vector.tensor_tensor(out=mc, in0=mc, in1=mc, op=ALU.mult)
                varcB = s2cB
                nc.vector.scalar_tensor_tensor(
                    out=varcB, in0=s2cB, scalar=1.0 / P, in1=mc,
                    op0=ALU.mult, op1=ALU.subtract)
                dotp = ps2.tile([LC, L], f32, tag="mid")
                nc.tensor.matmul(dotp, rowp[b * 64:(b + 1) * 64, :],
                                 colp[b * 64:(b + 1) * 64, :],
                                 tile_position=(b * 64, 0))
                t0 = smal.tile([LC, L], f32, tag="t0i", bufs=1)
                nc.vector.tensor_scalar_mul(out=t0, in0=dotp, scalar1=1.0 / 32.0)
                nc.vector.scalar_tensor_tensor(
                    out=t0, in0=mcB, scalar=m2r, in1=t0, op0=ALU.mult, op1=ALU.add)
                nc.vector.scalar_tensor_tensor(
                    out=t0, in0=varcB, scalar=varr, in1=t0,
                    op0=ALU.add, op1=ALU.add)
                nc.scalar.activation(t0, t0, AF.Sqrt, bias=epsv[0:LC, :], scale=1.0)
                nc.vector.reciprocal(t0, t0)
                ib = smal.tile([LC, L], bf16, tag="ibi", bufs=1)
                nc.vector.tensor_copy(ib, t0)
                nc.sync.dma_start(istd_dram[b:b + 1, :], ib)

            # ---- phase E: pair update ----
            for grp in range(NGRP):
                u = run.tile([128, SL], bf16, tag="useg")
                for k4 in range(GRP):
                    io = grp * GRP + k4
                    iB = run.tile([128, L], bf16, tag="istdB")
                    nc.sync.dma_start(
                        iB[0:64, :], pbc(istd_dram[0:1, io * L:(io + 1) * L], 64))
                    nc.sync.dma_start(
                        iB[64:128, :], pbc(istd_dram[1:2, io * L:(io + 1) * L], 64))
                    nc.vector.scalar_tensor_tensor(
                        out=u[:, k4 * L:(k4 + 1) * L], in0=qcol,
                        scalar=qrow[:, io:io + 1], in1=iB,
                        op0=ALU.add, op1=ALU.mult)
                g = run.tile([128, SL], bf16, tag="gseg")
                nc.scalar.activation(g, u, AF.Gelu)
                for c3 in range(3):
                    c0 = grp * SL + c3 * CH
                    pu = ps.tile([128, CH], f32, tag="big")
                    quad_mm(pu, d["pw2"], g[:, c3 * CH:(c3 + 1) * CH],
                            lslice=slice(0, P))
                    nc.vector.scalar_tensor_tensor(
                        out=pair[:, c0:c0 + CH], in0=pu, scalar=d["vecs"][:, 0:1],
                        in1=pair[:, c0:c0 + CH], op0=ALU.add, op1=ALU.add)

        # ---- distogram / recycle ----
        stats_pass(lw[0]["biasS"])
        for b in range(2):
            istd_from_stats(b, want_fix=False, want_dram=True)
        final = cyc == N_REC - 1
        wmat = dw2_sb if final else dw2r_sb
        for ch in range(NCH):
            c0 = ch * CH
            wp = ps.tile([128, CH], f32, tag="big")
            pbf = run.tile([128, CH], bf16, tag="pbf")
            nc.vector.tensor_copy(pbf, pair[:, c0:c0 + CH])
            nc.tensor.matmul(wp[0:64, :], dm1_sb[0:64, :], pbf[0:64, :],
                             tile_position=(0, 0))
            nc.tensor.matmul(wp[64:128, :], dm1_sb[64:128, :], pbf[64:128, :],
                             tile_position=(64, 64))
            iB = run.tile([128, CH], bf16, tag="istdBd")
            nc.sync.dma_start(iB[0:64, :], pbc(istd_dram[0:1, c0:c0 + CH], 64))
            nc.sync.dma_start(iB[64:128, :], pbc(istd_dram[1:2, c0:c0 + CH], 64))
            ud = run.tile([128, CH], bf16, tag="ud")
            nc.vector.tensor_tensor(out=ud, in0=wp, in1=iB, op=ALU.mult)
            gd = run.tile([128, CH], bf16, tag="gd")
            nc.scalar.activation(gd, ud, AF.Gelu)
            tp = ps.tile([128, CH], f32, tag="big")
            quad_mm(tp, wmat, gd)
            if final:
                ts_ = run.tile([128, CH], f32, tag="tsb")
                nc.vector.tensor_copy(ts_, tp)
                nc.sync.dma_start(t_out[:, c0:c0 + CH], ts_)
            else:
                nc.vector.scalar_tensor_tensor(
                    out=pair[:, c0:c0 + CH], in0=tp, scalar=dvec_sb[:, 0:1],
                    in1=pair[:, c0:c0 + CH], op0=ALU.add, op1=ALU.add)
                rb = run.tile([128, CH], bf16, tag="rbf")
                nc.vector.tensor_copy(rb, tp)
                nc.sync.dma_start(r_dram[:, c0:c0 + CH], rb)
        if not final:
            rv = r_dram[:, :].rearrange("p (i j) -> p i j", i=LC)
            for dd in range(NC):
                nc.sync.dma_start(a2a_in[dd, :, :],
                                  rv[:, :, dd * LC:(dd + 1) * LC])
            nc.gpsimd.collective_compute(
                "AllToAll", ALU.bypass, replica_groups=RG,
                ins=[a2a_in[:, :, :]], outs=[a2a_out[:, :, :]])
            pv = pair[:, :].rearrange("p (i j) -> p i j", i=LC)
            for dd in range(NC):
                rc = run.tile([128, LC, LC], bf16, tag="rrc")
                nc.sync.dma_start(rc, a2a_out[dd, :, :])
                nc.vector.tensor_tensor(
                    out=pv[:, :, dd * LC:(dd + 1) * LC],
                    in0=pv[:, :, dd * LC:(dd + 1) * LC],
                    in1=rc.transpose([0, 2, 1]), op=ALU.add)

    ctx.close()


# ============================================================================
# kernel() entry
# ============================================================================

def kernel(x, params):
    from concourse.bass_utils import run_bass_kernel_spmd

    f = _fold_params(params)
    hp = _host_precompute(x, f)

    if "nc" not in _CACHE:
        _CACHE["nc"] = _build_program(f)
    nc = _CACHE["nc"]

    in_maps = _make_in_maps(f, hp)
    res = run_bass_kernel_spmd(nc, in_maps, core_ids=list(range(NC)))
    _CACHE["last_res"] = res

    t = np.zeros((B, L, L, NB), np.float32)
    for c in range(NC):
        o = res.results[c]["t_out"].reshape(128, LC, L)
        for b in range(B):
            t[b, c * LC:(c + 1) * LC] = np.transpose(o[b * NB:(b + 1) * NB],
                                                     (1, 2, 0))
    t = t + f["db2"]
    t = (t + np.swapaxes(t, 1, 2)) / 2.0
    return t.astype(np.float32)


def _make_in_maps(f, hp):
    base = {}
    base["colp0"] = _bp(hp["colp0"])
    base["onesbp"] = _bf(np.ones((128, 1), np.float32))
    for li, d in enumerate(f["layers"]):
        base[f"wq_{li}"] = _bf(d["Wq"])
        base[f"wk_{li}"] = _bf(d["Wk"])
        base[f"wv_{li}"] = _bf(d["Wv"])
        base[f"wo_{li}"] = _bf(d["Wo"])
        base[f"w1_{li}"] = _bf(d["W1"])
        base[f"w2_{li}"] = _bf(d["W2"])
        base[f"wi_{li}"] = _bf(d["Wi"])
        base[f"wj_{li}"] = _bf(d["Wj"])
        base[f"m1_{li}"] = _bf(np.concatenate([d["M1"], d["M1"]], 0))
        base[f"pw2_{li}"] = _bf(np.concatenate([d["pW2"], d["pW2"]], 0))
        bS = np.zeros((128, 8), np.float32)
        bS[0:64, 0:4] = d["Wbp"]
        bS[0:64, 4] = 1.0
        bS[64:128] = bS[0:64]
        base[f"biasS_{li}"] = _bf(bS)
        w1c = np.zeros((128, 8), np.float32)
        for mo in range(8):
            w1c[:, mo] = d["W1_c"][mo * 128:(mo + 1) * 128]
        base[f"w1c_{li}"] = w1c
        vec = np.zeros((128, 8), np.float32)
        vec[0:64, 0] = d["pb2"]
        vec[64:128, 0] = d["pb2"]
        vec[0:64, 1] = d["bout"]
        vec[64:128, 1] = d["bout"]
        base[f"vecs_{li}"] = vec
    base["dm1"] = _bf(np.concatenate([f["dM1"], f["dM1"]], 0))
    base["dw2"] = _bf(np.concatenate([f["dW2"], f["dW2"]], 0))
    base["dw2r"] = _bf(np.concatenate([f["dW2r"], f["dW2r"]], 0))
    dvec = np.zeros((128, 4), np.float32)
    dvec[0:64, 0] = f["drc"]
    dvec[64:128, 0] = f["drc"]
    base["dvec"] = dvec

    h = hp["h_init"]
    hall = np.zeros((96, NC + 1, H), np.float32)
    for r in range(NC):
        for b in range(B):
            hall[b * LC:(b + 1) * LC, 1 + r] = h[b, r * LC:(r + 1) * LC]

    Tbp = np.concatenate([hp["T"].T, hp["T"].T], 0).astype(np.float32)

    maps = []
    for c in range(NC):
        m = dict(base)
        h0 = hall.copy()
        h0[:, 0] = hall[:, 1 + c]
        m["h0"] = h0
        m["rowp0"] = _bp(hp["rowp0"][:, c * LC:(c + 1) * LC])
        lo = L - 1 - (c * LC + LC - 1)
        m["Tc"] = np.ascontiguousarray(Tbp[:, lo:lo + LC - 1 + L])
        maps.append(m)
    return maps
